# revision 13
# baseline (speedup 1.0000x reference)
"""Trainium2 Bass kernel: GatedRecurrentCell (v4).

Math (per batch b, channel i, time t):
    pa = x @ Wa^T + ba ; pi = x @ Wi^T + bi
    a  = sigmoid(gate) * 3**(-sigmoid(pa))
    c  = sqrt(1-a^2) * silu(pi + bi)
    h_t = a_t*h_{t-1} + c_t   (h_{-1} = 0);  out = h

Key trick: 3**(-sigmoid(p)) == FA - FB*tanh(FC*p + FD) to 5.5e-4 abs
(global least-squares fit), so  a = aA + nB*tanh(FC*pa + tb)  with
per-channel aA = sigmoid(gate)*FA, nB = -sigmoid(gate)*FB, tb = FC*ba+FD.
This removes the Exp ACT pass and its table set entirely: the scalar
engine only runs {Silu,Tanh} (silu_and_others set) and {Sqrt}
(sqrt_and_others set) - 2 table loads per chunk-group instead of 3 - and
the decay computation becomes a cheap tensor_scalar affine on DVE/GPSIMD.

Mapping: data-parallel over batch (8 cores, 1 batch each); channels on
partitions (16 chunks of 128), time on the free dim. GEMMs in bf16.
The recurrence runs as tensor_tensor_scan on DVE over PAIRS of chunks
concatenated on the free dim (a[pair-start] is zeroed, restarting the
recurrence), which amortizes the scan instruction overhead. h is written
back in bf16 and upcast on the host.
"""

import functools
import os

import numpy as np

B, S, D, I = 8, 2048, 512, 2048
P = 128
NCORES = 8

# fit of 3^(-sigmoid(p)) = FA - FB*tanh(FC*p + FD), max abs err 5.5e-4
FA = 0.66661083
FB = 0.33324857
FC = 0.5096609
FD = 0.27426951

# knobs: how many of the 16 chunks run each flexible op on GPSIMD
AFF_GP_N = int(os.environ.get("GRC_AFF_GP", "10"))   # a = nB*th + aA
A2_GP_N = int(os.environ.get("GRC_A2_GP", "16"))     # a2 = a*a
CMUL_GP_N = int(os.environ.get("GRC_CMUL_GP", "0"))  # c = q*w
CW = 1024                                            # PSUM supertile width


def _gp_set(n, ni):
    """n of ni chunks, spread evenly."""
    if n <= 0:
        return set()
    return {int(round(j * ni / n)) % ni for j in range(n)}


def _build_nc(s, d, i, silu=True):
    import concourse.bacc as bacc
    import concourse.mybir as mybir
    import concourse.tile as tile
    from concourse.tile import add_dep_helper
    from contextlib import ExitStack

    F32 = mybir.dt.float32
    BF16 = mybir.dt.bfloat16
    AF = mybir.ActivationFunctionType
    ALU = mybir.AluOpType

    nd = d // P            # contraction chunks
    ni = i // P            # channel chunks
    cw = min(CW, s)
    nh = s // cw           # supertiles per channel row
    nmm = cw // 512        # matmuls (N=512) per supertile

    aff_gp = _gp_set(min(AFF_GP_N, ni), ni)
    a2_gp = _gp_set(min(A2_GP_N, ni), ni)
    cmul_gp = _gp_set(min(CMUL_GP_N, ni), ni)

    # chunk pairing for concatenated scans: (0,1),(2,3),... last two single
    n_single = 2 if ni >= 4 else 0
    paired = ni - n_single

    nc = bacc.Bacc("TRN2", target_bir_lowering=False, debug=False,
                   num_devices=NCORES)

    xT_d = nc.dram_tensor("xT", [d, s], BF16, kind="ExternalInput").ap()
    waT_d = nc.dram_tensor("WaT", [ni, P, d], BF16, kind="ExternalInput").ap()
    wiT_d = nc.dram_tensor("WiT", [ni, P, d], BF16, kind="ExternalInput").ap()
    aA_d = nc.dram_tensor("aA", [P, ni], F32, kind="ExternalInput").ap()
    nB_d = nc.dram_tensor("nB", [P, ni], F32, kind="ExternalInput").ap()
    tb_d = nc.dram_tensor("tb", [P, ni], F32, kind="ExternalInput").ap()
    sb_d = nc.dram_tensor("sb", [P, ni], F32, kind="ExternalInput").ap()
    out_d = nc.dram_tensor("out", [i, s], BF16, kind="ExternalOutput").ap()

    with tile.TileContext(nc) as tc:
        with ExitStack() as ctx:
            const_pool = ctx.enter_context(tc.tile_pool(name="const", bufs=1))
            xt_pool = ctx.enter_context(tc.tile_pool(name="xt", bufs=1))
            wst_pool = ctx.enter_context(tc.tile_pool(name="wst", bufs=1))
            ps_pool = ctx.enter_context(
                tc.tile_pool(name="mmpsum", bufs=1, space="PSUM"))
            rows = ctx.enter_context(tc.tile_pool(name="rows", bufs=1))

            aA_t = const_pool.tile([P, ni], F32, name="aA_t")
            nc.sync.dma_start(aA_t[:], aA_d[:])
            nB_t = const_pool.tile([P, ni], F32, name="nB_t")
            nc.sync.dma_start(nB_t[:], nB_d[:])
            tb_t = const_pool.tile([P, ni], F32, name="tb_t")
            nc.sync.dma_start(tb_t[:], tb_d[:])
            sb_t = const_pool.tile([P, ni], F32, name="sb_t")
            nc.sync.dma_start(sb_t[:], sb_d[:])

            # weight stream tiles; group 0 issued before the bulk x loads
            w_sbs = {}

            def load_weights(ic):
                wi_sb = wst_pool.tile([P, d], BF16, name=f"wi{ic}", tag="wi",
                                      bufs=3)
                nc.sync.dma_start(wi_sb[:], wiT_d[ic])
                wa_sb = wst_pool.tile([P, d], BF16, name=f"wa{ic}", tag="wa",
                                      bufs=3)
                nc.sync.dma_start(wa_sb[:], waT_d[ic])
                w_sbs[ic] = (wi_sb, wa_sb)

            # resident x^T; first column-block (all k) first, weights for
            # chunk 0/1 interleaved so the first GEMMs start ASAP
            xT_sb = [xt_pool.tile([P, s], BF16, name=f"xT{k}") for k in
                     range(nd)]
            for k in range(nd):
                nc.sync.dma_start(xT_sb[k][:, 0:cw], xT_d[k * P:(k + 1) * P,
                                                          0:cw])
            load_weights(0)
            load_weights(1)
            for h in range(1, nh):
                for k in range(nd):
                    nc.sync.dma_start(
                        xT_sb[k][:, h * cw:(h + 1) * cw],
                        xT_d[k * P:(k + 1) * P, h * cw:(h + 1) * cw])

            act_chain = []

            def act(out_ap, in_ap, func, **kw):
                inst = nc.scalar.activation(out_ap, in_ap, func, **kw)
                if act_chain:
                    add_dep_helper(inst.ins, act_chain[-1].ins, False,
                                   "act table phase order")
                act_chain.append(inst)
                return inst

            def gemm(ps, w_sb, h):
                for k in range(nd):
                    for m in range(nmm):
                        lo = h * cw + m * 512
                        nc.tensor.matmul(
                            ps[:, m * 512:(m + 1) * 512],
                            w_sb[:, k * P:(k + 1) * P],
                            xT_sb[k][:, lo:lo + 512],
                            start=(k == 0), stop=(k == nd - 1))

            state = {}   # ic -> dict of tiles

            def phase_silu_tanh(ics):
                for ic in ics:
                    if ic not in w_sbs:
                        load_weights(ic)
                    wi_sb, wa_sb = w_sbs.pop(ic)

                    w_t = rows.tile([P, s], BF16, name=f"w{ic}", tag="w",
                                    bufs=5)
                    th_t = rows.tile([P, s], F32, name=f"th{ic}", tag="th",
                                     bufs=3)
                    if ic < paired:
                        if ic % 2 == 0:
                            ap_t = rows.tile([P, 2 * s], F32, name=f"a{ic}",
                                             tag="ap", bufs=2)
                            state[ic + 1] = {"apair": ap_t}
                        else:
                            ap_t = state[ic]["apair"]
                        a_v = ap_t[:, (ic % 2) * s:(ic % 2 + 1) * s]
                    else:
                        ap_t = rows.tile([P, s], F32, name=f"a{ic}",
                                         tag="as", bufs=1)
                        a_v = ap_t[:]
                    a2_t = rows.tile([P, s], F32, name=f"a2{ic}", tag="a2",
                                     bufs=5)
                    st = state.setdefault(ic, {})
                    st.update({"w": w_t, "apair": ap_t, "a": a_v,
                               "a2": a2_t})

                    for h in range(nh):
                        sl = slice(h * cw, (h + 1) * cw)
                        pi_ps = ps_pool.tile([P, cw], F32, name=f"pi{ic}_{h}",
                                             tag="pi", bufs=2)
                        gemm(pi_ps, wi_sb, h)
                        if silu:
                            act(w_t[:, sl], pi_ps[:], AF.Silu,
                                bias=sb_t[:, ic:ic + 1])
                        else:
                            # CoreSim fallback: silu = sigmoid(z)*z
                            sg = rows.tile([P, cw], F32, name=f"sg{ic}_{h}",
                                           tag="sg", bufs=3)
                            act(sg[:], pi_ps[:], AF.Sigmoid,
                                bias=sb_t[:, ic:ic + 1])
                            pib = rows.tile([P, cw], F32, name=f"pib{ic}_{h}",
                                            tag="pib", bufs=3)
                            act(pib[:], pi_ps[:], AF.Identity,
                                bias=sb_t[:, ic:ic + 1])
                            nc.vector.tensor_mul(w_t[:, sl], sg[:], pib[:])
                    for h in range(nh):
                        sl = slice(h * cw, (h + 1) * cw)
                        pa_ps = ps_pool.tile([P, cw], F32, name=f"pa{ic}_{h}",
                                             tag="pa", bufs=2)
                        gemm(pa_ps, wa_sb, h)
                        act(th_t[:, sl], pa_ps[:], AF.Tanh,
                            scale=FC, bias=tb_t[:, ic:ic + 1])
                    # a = nB*th + aA, then a2 = a*a
                    aff_eng = nc.gpsimd if ic in aff_gp else nc.vector
                    aff_eng.tensor_scalar(
                        a_v, th_t[:], nB_t[:, ic:ic + 1],
                        aA_t[:, ic:ic + 1], op0=ALU.mult, op1=ALU.add)
                    a2_eng = nc.gpsimd if ic in a2_gp else nc.vector
                    a2_eng.tensor_mul(a2_t[:], a_v, a_v)
                    if ic < paired and ic % 2 == 1:
                        # restart the recurrence at the pair boundary
                        # (a[s] is never read by the scan: h = a*0 + c)
                        nc.gpsimd.memset(st["apair"][:, s:s + 1], 0.0)

            def phase_sqrt_scan(ics):
                for ic in ics:
                    st = state.pop(ic)
                    q_t = rows.tile([P, s], BF16, name=f"q{ic}", tag="q",
                                    bufs=3)
                    act(q_t[:], st["a2"][:], AF.Sqrt, scale=-1.0, bias=1.0)
                    if ic < paired:
                        if ic % 2 == 0:
                            cp_t = rows.tile([P, 2 * s], BF16, name=f"c{ic}",
                                             tag="cp", bufs=2)
                            state.setdefault(ic + 1, {})["cpair"] = cp_t
                        else:
                            cp_t = st["cpair"]
                        c_v = cp_t[:, (ic % 2) * s:(ic % 2 + 1) * s]
                    else:
                        cp_t = rows.tile([P, s], BF16, name=f"c{ic}",
                                         tag="cs", bufs=1)
                        c_v = cp_t[:]
                    cm_eng = nc.gpsimd if ic in cmul_gp else nc.vector
                    cm_eng.tensor_mul(c_v, q_t[:], st["w"][:])

                    if ic < paired and ic % 2 == 0:
                        state.setdefault(ic + 1, {})["half"] = (st["apair"],
                                                               cp_t)
                        continue
                    if ic < paired:
                        ap_t, cp_full = st["apair"], cp_t
                        h_t = rows.tile([P, 2 * s], BF16, name=f"h{ic}",
                                        tag="hp", bufs=2)
                        nc.vector.tensor_tensor_scan(
                            h_t[:], ap_t[:], cp_full[:], 0.0,
                            op0=ALU.mult, op1=ALU.add)
                        nc.sync.dma_start(out_d[(ic - 1) * P:ic * P, :],
                                          h_t[:, 0:s])
                        nc.sync.dma_start(out_d[ic * P:(ic + 1) * P, :],
                                          h_t[:, s:2 * s])
                    else:
                        h_t = rows.tile([P, s], BF16, name=f"h{ic}",
                                        tag="hs", bufs=1)
                        nc.vector.tensor_tensor_scan(
                            h_t[:], st["apair"][:], cp_t[:], 0.0,
                            op0=ALU.mult, op1=ALU.add)
                        nc.sync.dma_start(out_d[ic * P:(ic + 1) * P, :],
                                          h_t[:])

            # groups of 4 (pair-aligned), then a pair group, then singles
            groups = []
            g0 = 0
            while g0 < paired:
                gw = min(4, paired - g0)
                groups.append(list(range(g0, g0 + gw)))
                g0 += gw
            for ic in range(paired, ni):
                groups.append([ic])

            for ics in groups:
                phase_silu_tanh(ics)
                phase_sqrt_scan(ics)

    nc.compile()
    return nc


@functools.lru_cache(maxsize=2)
def _get_nc(s=S, d=D, i=I):
    return _build_nc(s, d, i)


LAST_RESULTS = None


def _prep_core_inputs(xb, shared):
    import ml_dtypes
    xT = np.ascontiguousarray(xb.T).astype(ml_dtypes.bfloat16)
    m = {"xT": xT}
    m.update(shared)
    return m


def _prep_shared(Wa, ba, Wi, bi, gate, d, i):
    import ml_dtypes
    ni = i // P
    nd = d // P
    # WaT[ic, p, k*128+j] = Wa[ic*128+j, k*128+p]  (lhsT blocks)
    WaT = np.ascontiguousarray(
        Wa.reshape(ni, P, nd, P).transpose(0, 3, 2, 1).reshape(ni, P, d)
    ).astype(ml_dtypes.bfloat16)
    WiT = np.ascontiguousarray(
        Wi.reshape(ni, P, nd, P).transpose(0, 3, 2, 1).reshape(ni, P, d)
    ).astype(ml_dtypes.bfloat16)
    alpha = 1.0 / (1.0 + np.exp(-gate.astype(np.float64)))
    aA = np.ascontiguousarray((alpha * FA).astype(np.float32).reshape(ni, P).T)
    nB = np.ascontiguousarray((-alpha * FB).astype(np.float32).reshape(ni, P).T)
    tb = np.ascontiguousarray(
        (FC * ba.astype(np.float64) + FD).astype(np.float32).reshape(ni, P).T)
    sb = np.ascontiguousarray(bi.astype(np.float32).reshape(ni, P).T)
    return {"WaT": WaT, "WiT": WiT, "aA": aA, "nB": nB, "tb": tb, "sb": sb}


def kernel(x, Wa, ba, Wi, bi, gate):
    global LAST_RESULTS
    from concourse.bass_utils import run_bass_kernel_spmd

    x = np.asarray(x, dtype=np.float32)
    b, s, d = x.shape
    i = Wa.shape[0]
    nc = _get_nc(s, d, i)

    shared = _prep_shared(
        np.asarray(Wa, np.float32), np.asarray(ba, np.float32),
        np.asarray(Wi, np.float32), np.asarray(bi, np.float32),
        np.asarray(gate, np.float32), d, i)

    in_maps = [_prep_core_inputs(x[bb], shared) for bb in range(b)]
    res = run_bass_kernel_spmd(nc, in_maps, list(range(b)))
    LAST_RESULTS = res
    out = np.stack([np.asarray(res.results[bb]["out"]).astype(np.float32).T
                    for bb in range(b)], axis=0)
    return np.ascontiguousarray(out)


# revision 15
# speedup vs baseline: 1.0480x; 1.0480x over previous
"""Trainium2 Bass kernel: GatedRecurrentCell (v4).

Math (per batch b, channel i, time t):
    pa = x @ Wa^T + ba ; pi = x @ Wi^T + bi
    a  = sigmoid(gate) * 3**(-sigmoid(pa))
    c  = sqrt(1-a^2) * silu(pi + bi)
    h_t = a_t*h_{t-1} + c_t   (h_{-1} = 0);  out = h

Key trick: 3**(-sigmoid(p)) == FA - FB*tanh(FC*p + FD) to 5.5e-4 abs
(global least-squares fit), so  a = aA + nB*tanh(FC*pa + tb)  with
per-channel aA = sigmoid(gate)*FA, nB = -sigmoid(gate)*FB, tb = FC*ba+FD.
This removes the Exp ACT pass and its table set entirely: the scalar
engine only runs {Silu,Tanh} (silu_and_others set) and {Sqrt}
(sqrt_and_others set) - 2 table loads per chunk-group instead of 3 - and
the decay computation becomes a cheap tensor_scalar affine on DVE/GPSIMD.

Mapping: data-parallel over batch (8 cores, 1 batch each); channels on
partitions (16 chunks of 128), time on the free dim. GEMMs in bf16.
The recurrence runs as tensor_tensor_scan on DVE over PAIRS of chunks
concatenated on the free dim (a[pair-start] is zeroed, restarting the
recurrence), which amortizes the scan instruction overhead. h is written
back in bf16 and upcast on the host.
"""

import functools
import os

import numpy as np

B, S, D, I = 8, 2048, 512, 2048
P = 128
NCORES = 8

# fit of 3^(-sigmoid(p)) = FA - FB*tanh(FC*p + FD), max abs err 5.5e-4
FA = 0.66661083
FB = 0.33324857
FC = 0.5096609
FD = 0.27426951

# knobs: how many of the 16 chunks run each flexible op on GPSIMD
AFF_GP_N = int(os.environ.get("GRC_AFF_GP", "10"))   # a = nB*th + aA
A2_GP_N = int(os.environ.get("GRC_A2_GP", "16"))     # a2 = a*a
CMUL_GP_N = int(os.environ.get("GRC_CMUL_GP", "0"))  # c = q*w
CW = 1024                                            # PSUM supertile width


def _gp_set(n, ni):
    """n of ni chunks, spread evenly."""
    if n <= 0:
        return set()
    return {int(round(j * ni / n)) % ni for j in range(n)}


def _build_nc(s, d, i, silu=True):
    import concourse.bacc as bacc
    import concourse.mybir as mybir
    import concourse.tile as tile
    from concourse.tile import add_dep_helper
    from contextlib import ExitStack

    F32 = mybir.dt.float32
    BF16 = mybir.dt.bfloat16
    AF = mybir.ActivationFunctionType
    ALU = mybir.AluOpType

    nd = d // P            # contraction chunks
    ni = i // P            # channel chunks
    cw = min(CW, s)
    nh = s // cw           # supertiles per channel row
    nmm = cw // 512        # matmuls (N=512) per supertile

    aff_gp = _gp_set(min(AFF_GP_N, ni), ni)
    a2_gp = _gp_set(min(A2_GP_N, ni), ni)
    cmul_gp = _gp_set(min(CMUL_GP_N, ni), ni)

    # chunk pairing for concatenated scans: (0,1),(2,3),... last two single
    n_single = 2 if ni >= 4 else 0
    paired = ni - n_single

    nc = bacc.Bacc("TRN2", target_bir_lowering=False, debug=False,
                   num_devices=NCORES)

    xT_d = nc.dram_tensor("xT", [d, s], BF16, kind="ExternalInput").ap()
    waT_d = nc.dram_tensor("WaT", [ni, P, d], BF16, kind="ExternalInput").ap()
    wiT_d = nc.dram_tensor("WiT", [ni, P, d], BF16, kind="ExternalInput").ap()
    aA_d = nc.dram_tensor("aA", [P, ni], F32, kind="ExternalInput").ap()
    nB_d = nc.dram_tensor("nB", [P, ni], F32, kind="ExternalInput").ap()
    tb_d = nc.dram_tensor("tb", [P, ni], F32, kind="ExternalInput").ap()
    sb_d = nc.dram_tensor("sb", [P, ni], F32, kind="ExternalInput").ap()
    out_d = nc.dram_tensor("out", [i, s], BF16, kind="ExternalOutput").ap()

    with tile.TileContext(nc) as tc:
        with ExitStack() as ctx:
            const_pool = ctx.enter_context(tc.tile_pool(name="const", bufs=1))
            xt_pool = ctx.enter_context(tc.tile_pool(name="xt", bufs=1))
            wst_pool = ctx.enter_context(tc.tile_pool(name="wst", bufs=1))
            ps_pool = ctx.enter_context(
                tc.tile_pool(name="mmpsum", bufs=1, space="PSUM"))
            rows = ctx.enter_context(tc.tile_pool(name="rows", bufs=1))

            aA_t = const_pool.tile([P, ni], F32, name="aA_t")
            nc.sync.dma_start(aA_t[:], aA_d[:])
            nB_t = const_pool.tile([P, ni], F32, name="nB_t")
            nc.sync.dma_start(nB_t[:], nB_d[:])
            tb_t = const_pool.tile([P, ni], F32, name="tb_t")
            nc.sync.dma_start(tb_t[:], tb_d[:])
            sb_t = const_pool.tile([P, ni], F32, name="sb_t")
            nc.sync.dma_start(sb_t[:], sb_d[:])

            # weight stream tiles; group 0 issued before the bulk x loads
            w_sbs = {}

            def load_weights(ic):
                wi_sb = wst_pool.tile([P, d], BF16, name=f"wi{ic}", tag="wi",
                                      bufs=3)
                nc.sync.dma_start(wi_sb[:], wiT_d[ic])
                wa_sb = wst_pool.tile([P, d], BF16, name=f"wa{ic}", tag="wa",
                                      bufs=3)
                nc.sync.dma_start(wa_sb[:], waT_d[ic])
                w_sbs[ic] = (wi_sb, wa_sb)

            # resident x^T; first column-block (all k) first, weights for
            # chunk 0/1 interleaved so the first GEMMs start ASAP
            xT_sb = [xt_pool.tile([P, s], BF16, name=f"xT{k}") for k in
                     range(nd)]
            for k in range(nd):
                nc.sync.dma_start(xT_sb[k][:, 0:cw], xT_d[k * P:(k + 1) * P,
                                                          0:cw])
            load_weights(0)
            load_weights(1)
            for h in range(1, nh):
                for k in range(nd):
                    nc.sync.dma_start(
                        xT_sb[k][:, h * cw:(h + 1) * cw],
                        xT_d[k * P:(k + 1) * P, h * cw:(h + 1) * cw])

            act_chain = []

            def act(out_ap, in_ap, func, **kw):
                inst = nc.scalar.activation(out_ap, in_ap, func, **kw)
                if act_chain:
                    add_dep_helper(inst.ins, act_chain[-1].ins, False,
                                   "act table phase order")
                act_chain.append(inst)
                return inst

            def gemm(ps, w_sb, h):
                for k in range(nd):
                    for m in range(nmm):
                        lo = h * cw + m * 512
                        nc.tensor.matmul(
                            ps[:, m * 512:(m + 1) * 512],
                            w_sb[:, k * P:(k + 1) * P],
                            xT_sb[k][:, lo:lo + 512],
                            start=(k == 0), stop=(k == nd - 1))

            state = {}   # ic -> dict of tiles

            def phase_silu_tanh(ics):
                for ic in ics:
                    if ic not in w_sbs:
                        load_weights(ic)
                    wi_sb, wa_sb = w_sbs.pop(ic)

                    w_t = rows.tile([P, s], BF16, name=f"w{ic}", tag="w",
                                    bufs=6)
                    th_t = rows.tile([P, s], F32, name=f"th{ic}", tag="th",
                                     bufs=3)
                    if ic < paired:
                        if ic % 2 == 0:
                            ap_t = rows.tile([P, 2 * s], F32, name=f"a{ic}",
                                             tag="ap", bufs=2)
                            state[ic + 1] = {"apair": ap_t}
                        else:
                            ap_t = state[ic]["apair"]
                        a_v = ap_t[:, (ic % 2) * s:(ic % 2 + 1) * s]
                    else:
                        ap_t = rows.tile([P, s], F32, name=f"a{ic}",
                                         tag="as", bufs=1)
                        a_v = ap_t[:]
                    a2_t = rows.tile([P, s], F32, name=f"a2{ic}", tag="a2",
                                     bufs=5)
                    st = state.setdefault(ic, {})
                    st.update({"w": w_t, "apair": ap_t, "a": a_v,
                               "a2": a2_t})

                    for h in range(nh):
                        sl = slice(h * cw, (h + 1) * cw)
                        pi_ps = ps_pool.tile([P, cw], F32, name=f"pi{ic}_{h}",
                                             tag="pi", bufs=2)
                        gemm(pi_ps, wi_sb, h)
                        if silu:
                            act(w_t[:, sl], pi_ps[:], AF.Silu,
                                bias=sb_t[:, ic:ic + 1])
                        else:
                            # CoreSim fallback: silu = sigmoid(z)*z
                            sg = rows.tile([P, cw], F32, name=f"sg{ic}_{h}",
                                           tag="sg", bufs=3)
                            act(sg[:], pi_ps[:], AF.Sigmoid,
                                bias=sb_t[:, ic:ic + 1])
                            pib = rows.tile([P, cw], F32, name=f"pib{ic}_{h}",
                                            tag="pib", bufs=3)
                            act(pib[:], pi_ps[:], AF.Identity,
                                bias=sb_t[:, ic:ic + 1])
                            nc.vector.tensor_mul(w_t[:, sl], sg[:], pib[:])
                    for h in range(nh):
                        sl = slice(h * cw, (h + 1) * cw)
                        pa_ps = ps_pool.tile([P, cw], F32, name=f"pa{ic}_{h}",
                                             tag="pa", bufs=2)
                        gemm(pa_ps, wa_sb, h)
                        act(th_t[:, sl], pa_ps[:], AF.Tanh,
                            scale=FC, bias=tb_t[:, ic:ic + 1])
                    # a = nB*th + aA, then a2 = a*a
                    aff_eng = nc.gpsimd if ic in aff_gp else nc.vector
                    aff_eng.tensor_scalar(
                        a_v, th_t[:], nB_t[:, ic:ic + 1],
                        aA_t[:, ic:ic + 1], op0=ALU.mult, op1=ALU.add)
                    a2_eng = nc.gpsimd if ic in a2_gp else nc.vector
                    a2_eng.tensor_mul(a2_t[:], a_v, a_v)
                    if ic < paired and ic % 2 == 1:
                        # restart the recurrence at the pair boundary
                        # (a[s] is never read by the scan: h = a*0 + c)
                        nc.gpsimd.memset(st["apair"][:, s:s + 1], 0.0)

            def phase_sqrt_scan(ics):
                for ic in ics:
                    st = state.pop(ic)
                    q_t = rows.tile([P, s], BF16, name=f"q{ic}", tag="q",
                                    bufs=3)
                    act(q_t[:], st["a2"][:], AF.Sqrt, scale=-1.0, bias=1.0)
                    if ic < paired:
                        if ic % 2 == 0:
                            cp_t = rows.tile([P, 2 * s], BF16, name=f"c{ic}",
                                             tag="cp", bufs=2)
                            state.setdefault(ic + 1, {})["cpair"] = cp_t
                        else:
                            cp_t = st["cpair"]
                        c_v = cp_t[:, (ic % 2) * s:(ic % 2 + 1) * s]
                    else:
                        cp_t = rows.tile([P, s], BF16, name=f"c{ic}",
                                         tag="cs", bufs=1)
                        c_v = cp_t[:]
                    cm_eng = nc.gpsimd if ic in cmul_gp else nc.vector
                    cm_eng.tensor_mul(c_v, q_t[:], st["w"][:])

                    if ic < paired and ic % 2 == 0:
                        state.setdefault(ic + 1, {})["half"] = (st["apair"],
                                                               cp_t)
                        continue
                    if ic < paired:
                        ap_t, cp_full = st["apair"], cp_t
                        h_t = rows.tile([P, 2 * s], BF16, name=f"h{ic}",
                                        tag="hp", bufs=2)
                        nc.vector.tensor_tensor_scan(
                            h_t[:], ap_t[:], cp_full[:], 0.0,
                            op0=ALU.mult, op1=ALU.add)
                        nc.sync.dma_start(out_d[(ic - 1) * P:ic * P, :],
                                          h_t[:, 0:s])
                        nc.sync.dma_start(out_d[ic * P:(ic + 1) * P, :],
                                          h_t[:, s:2 * s])
                    else:
                        h_t = rows.tile([P, s], BF16, name=f"h{ic}",
                                        tag="hs", bufs=1)
                        nc.vector.tensor_tensor_scan(
                            h_t[:], st["apair"][:], cp_t[:], 0.0,
                            op0=ALU.mult, op1=ALU.add)
                        nc.sync.dma_start(out_d[ic * P:(ic + 1) * P, :],
                                          h_t[:])

            # groups == scan pairs, then singles; lag-1 software pipeline
            # (sqrt/scan of group g runs after silu/tanh of group g+1, so
            # ACT never waits on the DVE/GPSIMD a-chain)
            groups = []
            g0 = 0
            while g0 < paired:
                gw = min(2, paired - g0)
                groups.append(list(range(g0, g0 + gw)))
                g0 += gw
            for ic in range(paired, ni):
                groups.append([ic])

            prev = None
            for ics in groups:
                phase_silu_tanh(ics)
                if prev is not None:
                    phase_sqrt_scan(prev)
                prev = ics
            phase_sqrt_scan(prev)

    nc.compile()
    return nc


@functools.lru_cache(maxsize=2)
def _get_nc(s=S, d=D, i=I):
    return _build_nc(s, d, i)


LAST_RESULTS = None


def _prep_core_inputs(xb, shared):
    import ml_dtypes
    xT = np.ascontiguousarray(xb.T).astype(ml_dtypes.bfloat16)
    m = {"xT": xT}
    m.update(shared)
    return m


def _prep_shared(Wa, ba, Wi, bi, gate, d, i):
    import ml_dtypes
    ni = i // P
    nd = d // P
    # WaT[ic, p, k*128+j] = Wa[ic*128+j, k*128+p]  (lhsT blocks)
    WaT = np.ascontiguousarray(
        Wa.reshape(ni, P, nd, P).transpose(0, 3, 2, 1).reshape(ni, P, d)
    ).astype(ml_dtypes.bfloat16)
    WiT = np.ascontiguousarray(
        Wi.reshape(ni, P, nd, P).transpose(0, 3, 2, 1).reshape(ni, P, d)
    ).astype(ml_dtypes.bfloat16)
    alpha = 1.0 / (1.0 + np.exp(-gate.astype(np.float64)))
    aA = np.ascontiguousarray((alpha * FA).astype(np.float32).reshape(ni, P).T)
    nB = np.ascontiguousarray((-alpha * FB).astype(np.float32).reshape(ni, P).T)
    tb = np.ascontiguousarray(
        (FC * ba.astype(np.float64) + FD).astype(np.float32).reshape(ni, P).T)
    sb = np.ascontiguousarray(bi.astype(np.float32).reshape(ni, P).T)
    return {"WaT": WaT, "WiT": WiT, "aA": aA, "nB": nB, "tb": tb, "sb": sb}


def kernel(x, Wa, ba, Wi, bi, gate):
    global LAST_RESULTS
    from concourse.bass_utils import run_bass_kernel_spmd

    x = np.asarray(x, dtype=np.float32)
    b, s, d = x.shape
    i = Wa.shape[0]
    nc = _get_nc(s, d, i)

    shared = _prep_shared(
        np.asarray(Wa, np.float32), np.asarray(ba, np.float32),
        np.asarray(Wi, np.float32), np.asarray(bi, np.float32),
        np.asarray(gate, np.float32), d, i)

    in_maps = [_prep_core_inputs(x[bb], shared) for bb in range(b)]
    res = run_bass_kernel_spmd(nc, in_maps, list(range(b)))
    LAST_RESULTS = res
    out = np.stack([np.asarray(res.results[bb]["out"]).astype(np.float32).T
                    for bb in range(b)], axis=0)
    return np.ascontiguousarray(out)


# revision 16
# speedup vs baseline: 1.0491x; 1.0010x over previous
"""Trainium2 Bass kernel: GatedRecurrentCell (v4).

Math (per batch b, channel i, time t):
    pa = x @ Wa^T + ba ; pi = x @ Wi^T + bi
    a  = sigmoid(gate) * 3**(-sigmoid(pa))
    c  = sqrt(1-a^2) * silu(pi + bi)
    h_t = a_t*h_{t-1} + c_t   (h_{-1} = 0);  out = h

Key trick: 3**(-sigmoid(p)) == FA - FB*tanh(FC*p + FD) to 5.5e-4 abs
(global least-squares fit), so  a = aA + nB*tanh(FC*pa + tb)  with
per-channel aA = sigmoid(gate)*FA, nB = -sigmoid(gate)*FB, tb = FC*ba+FD.
This removes the Exp ACT pass and its table set entirely: the scalar
engine only runs {Silu,Tanh} (silu_and_others set) and {Sqrt}
(sqrt_and_others set) - 2 table loads per chunk-group instead of 3 - and
the decay computation becomes a cheap tensor_scalar affine on DVE/GPSIMD.

Mapping: data-parallel over batch (8 cores, 1 batch each); channels on
partitions (16 chunks of 128), time on the free dim. GEMMs in bf16.
The recurrence runs as tensor_tensor_scan on DVE over PAIRS of chunks
concatenated on the free dim (a[pair-start] is zeroed, restarting the
recurrence), which amortizes the scan instruction overhead. h is written
back in bf16 and upcast on the host.
"""

import functools
import os

import numpy as np

B, S, D, I = 8, 2048, 512, 2048
P = 128
NCORES = 8

# fit of 3^(-sigmoid(p)) = FA - FB*tanh(FC*p + FD), max abs err 5.5e-4
FA = 0.66661083
FB = 0.33324857
FC = 0.5096609
FD = 0.27426951

# knobs: how many of the 16 chunks run each flexible op on GPSIMD
AFF_GP_N = int(os.environ.get("GRC_AFF_GP", "10"))   # a = nB*th + aA
A2_GP_N = int(os.environ.get("GRC_A2_GP", "16"))     # a2 = a*a
CMUL_GP_N = int(os.environ.get("GRC_CMUL_GP", "0"))  # c = q*w
CW = 1024                                            # PSUM supertile width


def _gp_set(n, ni):
    """n of ni chunks, spread evenly."""
    if n <= 0:
        return set()
    return {int(round(j * ni / n)) % ni for j in range(n)}


def _build_nc(s, d, i, silu=True):
    import concourse.bacc as bacc
    import concourse.mybir as mybir
    import concourse.tile as tile
    from concourse.tile import add_dep_helper
    from contextlib import ExitStack

    F32 = mybir.dt.float32
    BF16 = mybir.dt.bfloat16
    AF = mybir.ActivationFunctionType
    ALU = mybir.AluOpType

    nd = d // P            # contraction chunks
    ni = i // P            # channel chunks
    cw = min(CW, s)
    nh = s // cw           # supertiles per channel row
    nmm = cw // 512        # matmuls (N=512) per supertile

    aff_gp = _gp_set(min(AFF_GP_N, ni), ni)
    a2_gp = _gp_set(min(A2_GP_N, ni), ni)
    cmul_gp = _gp_set(min(CMUL_GP_N, ni), ni)

    # chunk pairing for concatenated scans: (0,1),(2,3),... last two single
    n_single = 2 if ni >= 4 else 0
    paired = ni - n_single

    nc = bacc.Bacc("TRN2", target_bir_lowering=False, debug=False,
                   num_devices=NCORES)

    xT_d = nc.dram_tensor("xT", [d, s], BF16, kind="ExternalInput").ap()
    waT_d = nc.dram_tensor("WaT", [ni, P, d], BF16, kind="ExternalInput").ap()
    wiT_d = nc.dram_tensor("WiT", [ni, P, d], BF16, kind="ExternalInput").ap()
    aA_d = nc.dram_tensor("aA", [P, ni], F32, kind="ExternalInput").ap()
    nB_d = nc.dram_tensor("nB", [P, ni], F32, kind="ExternalInput").ap()
    tb_d = nc.dram_tensor("tb", [P, ni], F32, kind="ExternalInput").ap()
    sb_d = nc.dram_tensor("sb", [P, ni], F32, kind="ExternalInput").ap()
    out_d = nc.dram_tensor("out", [i, s], BF16, kind="ExternalOutput").ap()

    with tile.TileContext(nc) as tc:
        with ExitStack() as ctx:
            const_pool = ctx.enter_context(tc.tile_pool(name="const", bufs=1))
            xt_pool = ctx.enter_context(tc.tile_pool(name="xt", bufs=1))
            wst_pool = ctx.enter_context(tc.tile_pool(name="wst", bufs=1))
            ps_pool = ctx.enter_context(
                tc.tile_pool(name="mmpsum", bufs=1, space="PSUM"))
            rows = ctx.enter_context(tc.tile_pool(name="rows", bufs=1))

            aA_t = const_pool.tile([P, ni], F32, name="aA_t")
            nc.sync.dma_start(aA_t[:], aA_d[:])
            nB_t = const_pool.tile([P, ni], F32, name="nB_t")
            nc.sync.dma_start(nB_t[:], nB_d[:])
            tb_t = const_pool.tile([P, ni], F32, name="tb_t")
            nc.sync.dma_start(tb_t[:], tb_d[:])
            sb_t = const_pool.tile([P, ni], F32, name="sb_t")
            nc.sync.dma_start(sb_t[:], sb_d[:])

            # weight stream tiles; group 0 issued before the bulk x loads
            w_sbs = {}

            def load_weights(ic):
                wi_sb = wst_pool.tile([P, d], BF16, name=f"wi{ic}", tag="wi",
                                      bufs=3)
                nc.sync.dma_start(wi_sb[:], wiT_d[ic])
                wa_sb = wst_pool.tile([P, d], BF16, name=f"wa{ic}", tag="wa",
                                      bufs=3)
                nc.sync.dma_start(wa_sb[:], waT_d[ic])
                w_sbs[ic] = (wi_sb, wa_sb)

            # resident x^T; first column-block (all k) first, weights for
            # chunk 0/1 interleaved so the first GEMMs start ASAP
            xT_sb = [xt_pool.tile([P, s], BF16, name=f"xT{k}") for k in
                     range(nd)]
            for k in range(nd):
                nc.sync.dma_start(xT_sb[k][:, 0:cw], xT_d[k * P:(k + 1) * P,
                                                          0:cw])
            load_weights(0)
            load_weights(1)
            for h in range(1, nh):
                for k in range(nd):
                    nc.sync.dma_start(
                        xT_sb[k][:, h * cw:(h + 1) * cw],
                        xT_d[k * P:(k + 1) * P, h * cw:(h + 1) * cw])

            act_chain = []

            def act(out_ap, in_ap, func, **kw):
                inst = nc.scalar.activation(out_ap, in_ap, func, **kw)
                if act_chain:
                    add_dep_helper(inst.ins, act_chain[-1].ins, False,
                                   "act table phase order")
                act_chain.append(inst)
                return inst

            def gemm(ps, w_sb, h):
                for k in range(nd):
                    for m in range(nmm):
                        lo = h * cw + m * 512
                        nc.tensor.matmul(
                            ps[:, m * 512:(m + 1) * 512],
                            w_sb[:, k * P:(k + 1) * P],
                            xT_sb[k][:, lo:lo + 512],
                            start=(k == 0), stop=(k == nd - 1))

            state = {}   # ic -> dict of tiles

            def phase_silu_tanh(ics):
                for ic in ics:
                    if ic not in w_sbs:
                        load_weights(ic)
                    wi_sb, wa_sb = w_sbs.pop(ic)

                    w_t = rows.tile([P, s], BF16, name=f"w{ic}", tag="w",
                                    bufs=6)
                    th_t = rows.tile([P, s], F32, name=f"th{ic}", tag="th",
                                     bufs=3)
                    if ic < paired:
                        if ic % 2 == 0:
                            ap_t = rows.tile([P, 2 * s], BF16, name=f"a{ic}",
                                             tag="ap", bufs=2)
                            state[ic + 1] = {"apair": ap_t}
                        else:
                            ap_t = state[ic]["apair"]
                        a_v = ap_t[:, (ic % 2) * s:(ic % 2 + 1) * s]
                    else:
                        ap_t = rows.tile([P, s], BF16, name=f"a{ic}",
                                         tag="as", bufs=1)
                        a_v = ap_t[:]
                    a2_t = rows.tile([P, s], F32, name=f"a2{ic}", tag="a2",
                                     bufs=5)
                    st = state.setdefault(ic, {})
                    st.update({"w": w_t, "apair": ap_t, "a": a_v,
                               "a2": a2_t})

                    for h in range(nh):
                        sl = slice(h * cw, (h + 1) * cw)
                        pi_ps = ps_pool.tile([P, cw], F32, name=f"pi{ic}_{h}",
                                             tag="pi", bufs=2)
                        gemm(pi_ps, wi_sb, h)
                        if silu:
                            act(w_t[:, sl], pi_ps[:], AF.Silu,
                                bias=sb_t[:, ic:ic + 1])
                        else:
                            # CoreSim fallback: silu = sigmoid(z)*z
                            sg = rows.tile([P, cw], F32, name=f"sg{ic}_{h}",
                                           tag="sg", bufs=3)
                            act(sg[:], pi_ps[:], AF.Sigmoid,
                                bias=sb_t[:, ic:ic + 1])
                            pib = rows.tile([P, cw], F32, name=f"pib{ic}_{h}",
                                            tag="pib", bufs=3)
                            act(pib[:], pi_ps[:], AF.Identity,
                                bias=sb_t[:, ic:ic + 1])
                            nc.vector.tensor_mul(w_t[:, sl], sg[:], pib[:])
                    for h in range(nh):
                        sl = slice(h * cw, (h + 1) * cw)
                        pa_ps = ps_pool.tile([P, cw], F32, name=f"pa{ic}_{h}",
                                             tag="pa", bufs=2)
                        gemm(pa_ps, wa_sb, h)
                        act(th_t[:, sl], pa_ps[:], AF.Tanh,
                            scale=FC, bias=tb_t[:, ic:ic + 1])
                    # a = nB*th + aA, then a2 = a*a
                    aff_eng = nc.gpsimd if ic in aff_gp else nc.vector
                    aff_eng.tensor_scalar(
                        a_v, th_t[:], nB_t[:, ic:ic + 1],
                        aA_t[:, ic:ic + 1], op0=ALU.mult, op1=ALU.add)
                    a2_eng = nc.gpsimd if ic in a2_gp else nc.vector
                    a2_eng.tensor_mul(a2_t[:], a_v, a_v)
                    if ic < paired and ic % 2 == 1:
                        # restart the recurrence at the pair boundary
                        # (a[s] is never read by the scan: h = a*0 + c)
                        nc.gpsimd.memset(st["apair"][:, s:s + 1], 0.0)

            def phase_sqrt_scan(ics):
                for ic in ics:
                    st = state.pop(ic)
                    q_t = rows.tile([P, s], BF16, name=f"q{ic}", tag="q",
                                    bufs=3)
                    act(q_t[:], st["a2"][:], AF.Sqrt, scale=-1.0, bias=1.0)
                    if ic < paired:
                        if ic % 2 == 0:
                            cp_t = rows.tile([P, 2 * s], BF16, name=f"c{ic}",
                                             tag="cp", bufs=2)
                            state.setdefault(ic + 1, {})["cpair"] = cp_t
                        else:
                            cp_t = st["cpair"]
                        c_v = cp_t[:, (ic % 2) * s:(ic % 2 + 1) * s]
                    else:
                        cp_t = rows.tile([P, s], BF16, name=f"c{ic}",
                                         tag="cs", bufs=1)
                        c_v = cp_t[:]
                    cm_eng = nc.gpsimd if ic in cmul_gp else nc.vector
                    cm_eng.tensor_mul(c_v, q_t[:], st["w"][:])

                    if ic < paired and ic % 2 == 0:
                        state.setdefault(ic + 1, {})["half"] = (st["apair"],
                                                               cp_t)
                        continue
                    if ic < paired:
                        ap_t, cp_full = st["apair"], cp_t
                        h_t = rows.tile([P, 2 * s], BF16, name=f"h{ic}",
                                        tag="hp", bufs=2)
                        nc.vector.tensor_tensor_scan(
                            h_t[:], ap_t[:], cp_full[:], 0.0,
                            op0=ALU.mult, op1=ALU.add)
                        nc.sync.dma_start(out_d[(ic - 1) * P:ic * P, :],
                                          h_t[:, 0:s])
                        nc.sync.dma_start(out_d[ic * P:(ic + 1) * P, :],
                                          h_t[:, s:2 * s])
                    else:
                        h_t = rows.tile([P, s], BF16, name=f"h{ic}",
                                        tag="hs", bufs=1)
                        nc.vector.tensor_tensor_scan(
                            h_t[:], st["apair"][:], cp_t[:], 0.0,
                            op0=ALU.mult, op1=ALU.add)
                        nc.sync.dma_start(out_d[ic * P:(ic + 1) * P, :],
                                          h_t[:])

            # groups == scan pairs, then singles; lag-1 software pipeline
            # (sqrt/scan of group g runs after silu/tanh of group g+1, so
            # ACT never waits on the DVE/GPSIMD a-chain)
            groups = []
            g0 = 0
            while g0 < paired:
                gw = min(2, paired - g0)
                groups.append(list(range(g0, g0 + gw)))
                g0 += gw
            for ic in range(paired, ni):
                groups.append([ic])

            prev = None
            for ics in groups:
                phase_silu_tanh(ics)
                if prev is not None:
                    phase_sqrt_scan(prev)
                prev = ics
            phase_sqrt_scan(prev)

    nc.compile()
    return nc


@functools.lru_cache(maxsize=2)
def _get_nc(s=S, d=D, i=I):
    return _build_nc(s, d, i)


LAST_RESULTS = None


def _prep_core_inputs(xb, shared):
    import ml_dtypes
    xT = np.ascontiguousarray(xb.T).astype(ml_dtypes.bfloat16)
    m = {"xT": xT}
    m.update(shared)
    return m


def _prep_shared(Wa, ba, Wi, bi, gate, d, i):
    import ml_dtypes
    ni = i // P
    nd = d // P
    # WaT[ic, p, k*128+j] = Wa[ic*128+j, k*128+p]  (lhsT blocks)
    WaT = np.ascontiguousarray(
        Wa.reshape(ni, P, nd, P).transpose(0, 3, 2, 1).reshape(ni, P, d)
    ).astype(ml_dtypes.bfloat16)
    WiT = np.ascontiguousarray(
        Wi.reshape(ni, P, nd, P).transpose(0, 3, 2, 1).reshape(ni, P, d)
    ).astype(ml_dtypes.bfloat16)
    alpha = 1.0 / (1.0 + np.exp(-gate.astype(np.float64)))
    aA = np.ascontiguousarray((alpha * FA).astype(np.float32).reshape(ni, P).T)
    nB = np.ascontiguousarray((-alpha * FB).astype(np.float32).reshape(ni, P).T)
    tb = np.ascontiguousarray(
        (FC * ba.astype(np.float64) + FD).astype(np.float32).reshape(ni, P).T)
    sb = np.ascontiguousarray(bi.astype(np.float32).reshape(ni, P).T)
    return {"WaT": WaT, "WiT": WiT, "aA": aA, "nB": nB, "tb": tb, "sb": sb}


def kernel(x, Wa, ba, Wi, bi, gate):
    global LAST_RESULTS
    from concourse.bass_utils import run_bass_kernel_spmd

    x = np.asarray(x, dtype=np.float32)
    b, s, d = x.shape
    i = Wa.shape[0]
    nc = _get_nc(s, d, i)

    shared = _prep_shared(
        np.asarray(Wa, np.float32), np.asarray(ba, np.float32),
        np.asarray(Wi, np.float32), np.asarray(bi, np.float32),
        np.asarray(gate, np.float32), d, i)

    in_maps = [_prep_core_inputs(x[bb], shared) for bb in range(b)]
    res = run_bass_kernel_spmd(nc, in_maps, list(range(b)))
    LAST_RESULTS = res
    out = np.stack([np.asarray(res.results[bb]["out"]).astype(np.float32).T
                    for bb in range(b)], axis=0)
    return np.ascontiguousarray(out)
